# revision 13
# baseline (speedup 1.0000x reference)
"""Trainium2 Bass kernel for intra-segment KNN (K=64 neighbours + self).

Problem: coordinates [32768, 4] f32 split into 8 equal segments (events) of
4096 points; per point, find the 65 nearest points (incl. self) within its
segment, returning (idx int32 [32768,65], dist f32 [32768,65]) sorted by
ascending squared distance (matching jax.lax.top_k on -d2, ties to the
lower index).

Sharding: one event per NeuronCore (8 cores), pure data parallel.

Per-core algorithm (S=4096 points, D=4 dims), "packed-key" selection:
  - negkey[r, j] = -d2[r, j] via TensorE (fp32 for accuracy):
    psum = 2*c_r.c_j - |c_j|^2 (contraction 8: lhsT rows 0-3 = 2*c^T,
    rows 4-7 = -1; rhs rows 0-3 = c^T, rows 4-7 = (c^T)^2), then ScalarE
    adds the per-row bias -|c_r|^2 while copying PSUM -> SBUF (fp32).
  - GpSimd packs sort keys in one fused pass (scalar_tensor_tensor):
    key = (negkey & ~0x7F) | (j mod 128).  An fp32 compare of two keys
    orders by the value's top 25 bits and, on ties, prefers the lower
    column (keys <= 0), reproducing top_k tie-breaking to 2^-17 relative
    precision -- so selection needs no MaxIndex8 passes at all.
  - VectorE group phase per 128-row tile: 32 groups of 128 columns, Max8
    keeps the top-8 keys of each -> C [128, 256].  (P(a 128-col group
    holds >8 of the true top-65) ~ 2e-4 -- a handful of rows per run.)
  - GpSimd repacks C for the global phase: C2 = (C & ~0xFF) | slot
    (slot = position in C, encoding (group, rank)); also colf[slot] =
    (C & 0x7F) + 128*group = the candidate's column.
  - VectorE C phase: 9 rounds of (Max8 + MatchReplace8) over C2
    -> top-72 keys V, sorted; low byte of each key = its C slot.
  - GpSimd unscrambles columns without any gather op: scatter ranks into
    slot space (W[slot[k]] = k+1), decrement, then scatter columns into
    rank space (Y[W[q]-1] = colf[q]); Y[:, :65] = neighbour columns.
  - dist = Relu(-V) on ScalarE (top-24-bit d2, rel err ~2^-16).

VectorE does only 49 ops/tile (one grouped Max8 pass + a 256-wide
extraction); ScalarE, GpSimd, TensorE and DMA run in parallel under it.
"""

import numpy as np

S = 4096          # points per segment
D = 4             # coordinate dims
B = 8             # segments / cores
K1 = 65           # neighbours incl. self
P = 128           # partitions
NT = S // P       # 32 row tiles
GW = 128          # group width (columns per group)
NG = S // GW      # 32 groups
CW = NG * 8       # candidate array width (256)
NR = 9            # extraction rounds (9*8 = 72 >= 65)
RW = NR * 8       # 72
HB = 1024         # psum chunk width (2 banks)
NEG_BIG = -3.0e38 # "minus infinity" replacement; never equals a real key
LOCM = GW - 1     # local-column mask
SLOTM = CW - 1    # slot mask

_NC_CACHE = {}


def _build_nc():
    import concourse.bacc as bacc
    import concourse.mybir as mybir
    from concourse import bass, library_config
    from concourse.tile import TileContext

    fp32 = mybir.dt.float32
    f32r = mybir.dt.float32r
    u8 = mybir.dt.uint8
    i16 = mybir.dt.int16
    i32 = mybir.dt.int32
    Alu = mybir.AluOpType
    Act = mybir.ActivationFunctionType

    nc = bacc.Bacc(None, target_bir_lowering=False, debug=False)

    coords = nc.dram_tensor("coords", [S, D], fp32, kind="ExternalInput")
    out_dist = nc.dram_tensor("out_dist", [S, K1], fp32, kind="ExternalOutput")
    out_idx = nc.dram_tensor("out_idx", [S, K1], i32, kind="ExternalOutput")

    with TileContext(nc) as tc:
        with (
            tc.tile_pool(name="const", bufs=1) as cpool,
            tc.tile_pool(name="cand", bufs=2) as candpool,
            tc.tile_pool(name="small", bufs=3) as spool,
            tc.tile_pool(name="outs", bufs=3) as opool,
            tc.tile_pool(name="psum", bufs=3, space="PSUM") as ppool,
            tc.tile_pool(name="psumT", bufs=2, space="PSUM") as ptpool,
        ):
            # ---------------- persistent tensors ----------------
            rhs8 = cpool.tile([8, S], fp32)     # rows 0-3: c^T, rows 4-7: (c^T)^2
            lhsT8 = cpool.tile([8, S], fp32)    # rows 0-3: 2*c^T, rows 4-7: -1
            ident = cpool.tile([P, P], fp32)    # identity for PE transpose
            sqr_all = cpool.tile([P, NT], fp32) # -|c_r|^2 per row, per tile col
            nkA = cpool.tile([P, S], fp32)      # negkey/packed-key ping-pong
            nkB = cpool.tile([P, S], fp32)
            loc8 = cpool.tile([P, S], u8)       # j mod 128, as bytes
            slotb = cpool.tile([P, CW], u8)     # q (C-slot iota), as bytes
            goff = cpool.tile([P, CW], i16)     # 128 * (q >> 3)
            kio1 = cpool.tile([P, RW], i16)     # 1..72
            lscale = cpool.tile([8, 1], fp32)   # rows 0-3: 2.0, rows 4-7: 0.0
            lbias = cpool.tile([8, 1], fp32)    # rows 0-3: 0.0, rows 4-7: -1.0

            # per-row scale/bias used to build lhsT8 in one activation
            nc.vector.memset(lscale, 2.0)
            nc.gpsimd.affine_select(lscale, lscale, [[0, 1]], Alu.is_ge, 0.0,
                                    base=3, channel_multiplier=-1)
            nc.vector.memset(lbias, -1.0)
            nc.gpsimd.affine_select(lbias, lbias, [[0, 1]], Alu.is_ge, 0.0,
                                    base=-4, channel_multiplier=1)

            # identity matrix: ones masked to the diagonal
            nc.vector.memset(ident, 1.0)
            nc.gpsimd.affine_select(
                ident, ident, [[1, P]], Alu.is_equal, 0.0,
                base=0, channel_multiplier=-1,
            )
            nc.gpsimd.iota(loc8, [[0, NG], [1, GW]], base=0,
                           channel_multiplier=0,
                           allow_small_or_imprecise_dtypes=True)
            nc.gpsimd.iota(slotb, [[1, CW]], base=0, channel_multiplier=0,
                           allow_small_or_imprecise_dtypes=True)
            nc.gpsimd.iota(goff, [[GW, NG], [0, 8]], base=0, channel_multiplier=0)
            nc.gpsimd.iota(kio1, [[1, RW]], base=1, channel_multiplier=0)

            # ---------------- prologue: build c^T layout ----------------
            # (engine APs must start at partition 0, so lhsT8 is written in
            # ONE activation whose per-row scale/bias produce 2*c^T on rows
            # 0-3 and -1 on rows 4-7.)
            for t in range(NT):
                ct8 = spool.tile([P, 2 * D], fp32, tag="ct8")
                # cols 0-3 <- coords rows, cols 4-7 <- squares
                nc.sync.dma_start(ct8[:, 0:D], coords[t * P:(t + 1) * P, :])
                nc.scalar.activation(ct8[:, D:2 * D], ct8[:, 0:D], Act.Square)
                # -|c_r|^2 for this tile's 128 rows
                nc.vector.tensor_reduce(
                    sqr_all[:, t:t + 1], ct8[:, D:2 * D],
                    axis=mybir.AxisListType.X, op=Alu.add, negate=True,
                )
                # transpose [128, 8] -> [8, 128]
                pT = ptpool.tile([2 * D, P], fp32, tag="pT")
                nc.tensor.transpose(pT, ct8, ident)
                cs = slice(t * P, (t + 1) * P)
                nc.scalar.activation(rhs8[:, cs], pT, Act.Copy)
                nc.scalar.activation(lhsT8[:, cs], pT, Act.Identity,
                                     bias=lbias, scale=lscale)

            # local_scatter lives in gpsimd ucode library 7; load it once.
            # (iota/affine_select above ran in the default library; the
            # tensor_scalar / scalar_tensor_tensor ops below are built-ins.)
            nc.gpsimd.load_library(library_config.local_scatter)

            # ---------------- main loop over row tiles ----------------
            for t in range(NT):
                cs = slice(t * P, (t + 1) * P)
                nk = nkA if t % 2 == 0 else nkB
                for h in range(S // HB):
                    ps = ppool.tile([P, HB], fp32, tag="ps")
                    for m in range(HB // 512):
                        col0 = h * HB + m * 512
                        nc.tensor.matmul(
                            ps[:, m * 512:(m + 1) * 512],
                            lhsT8[:, cs],
                            rhs8[:, col0:col0 + 512],
                            start=True, stop=True,
                        )
                    # negkey = psum - |c_r|^2
                    nc.scalar.activation(
                        nk[:, h * HB:(h + 1) * HB], ps,
                        Act.Identity, bias=sqr_all[:, t:t + 1],
                    )
                # pack keys in place: low byte of each fp32 negkey is
                # overwritten with (j mod 128), leaving the value's top 24
                # bits as the sort key and the local column as tiebreaker.
                nc.gpsimd.tensor_copy(nk.bitcast(u8)[:, 0:4 * S:4], loc8)

                # ---- group phase: top-8 keys of each 128-wide group ----
                Cv = candpool.tile([P, CW], fp32, tag="Cv")
                for g in range(NG):
                    nc.vector.max(Cv[:, 8 * g:8 * g + 8],
                                  nk[:, g * GW:(g + 1) * GW])

                # slot-code C for the global phase: C2 = Cv with the low
                # byte replaced by the C-slot index q; colf[q] = column.
                C2 = candpool.tile([P, CW], fp32, tag="C2")
                locb = spool.tile([P, CW], i16, tag="locb")
                colf = spool.tile([P, CW], i16, tag="colf")
                nc.gpsimd.tensor_copy(C2, Cv)
                nc.gpsimd.tensor_copy(C2.bitcast(u8)[:, 0:4 * CW:4], slotb)
                nc.scalar.activation(locb, Cv.bitcast(u8)[:, 0:4 * CW:4],
                                     Act.Copy)
                nc.vector.tensor_tensor(out=colf, in0=locb, in1=goff,
                                        op=Alu.add)

                # ---- C phase: global top-72 keys of the 256 candidates ----
                V = spool.tile([P, RW], fp32, tag="V")
                for r in range(NR):
                    v8 = V[:, 8 * r:8 * r + 8]
                    nc.vector.max(v8, C2)
                    if r + 1 < NR:
                        nc.vector.match_replace(C2, v8, C2, NEG_BIG)

                # ---- column recovery (double local_scatter) ----
                slot72 = spool.tile([P, RW], i16, tag="slot72")
                nc.scalar.activation(slot72, V.bitcast(u8)[:, 0:4 * RW:4],
                                     Act.Copy)
                W = spool.tile([P, CW], i16, tag="W")
                nc.gpsimd.local_scatter(W, kio1, slot72,
                                        channels=P, num_elems=CW, num_idxs=RW)
                Wm = spool.tile([P, CW], i16, tag="Wm")
                nc.vector.tensor_scalar_add(Wm, W, -1)
                Y = spool.tile([P, RW], i16, tag="Y")
                nc.gpsimd.local_scatter(Y, colf, Wm,
                                        channels=P, num_elems=RW, num_idxs=CW)

                # ---- outputs ----
                dist65 = opool.tile([P, K1], fp32, tag="dist65")
                idx65 = opool.tile([P, K1], i32, tag="idx65")
                nc.scalar.activation(dist65, V[:, :K1], Act.Relu, scale=-1.0)
                nc.vector.tensor_copy(idx65, Y[:, :K1])
                nc.sync.dma_start(out_dist[cs, :], dist65)
                nc.sync.dma_start(out_idx[cs, :], idx65)

    nc.finalize()
    return nc


def _get_nc():
    if "nc" not in _NC_CACHE:
        _NC_CACHE["nc"] = _build_nc()
    return _NC_CACHE["nc"]


def _numpy_fallback(coordinates, row_splits):
    """Pure-numpy replica of the reference (used only on unexpected shapes)."""
    nB = int(row_splits.shape[0] - 1)
    N, nD = coordinates.shape
    nS = N // nB
    c = coordinates.reshape(nB, nS, nD).astype(np.float32)
    sq = np.sum(c * c, axis=-1)
    d2 = sq[:, :, None] + sq[:, None, :] - 2.0 * np.einsum(
        "bsd,btd->bst", c, c)
    d2 = np.maximum(d2, 0.0).astype(np.float32)
    k1 = min(K1, nS)
    idx = np.argsort(d2, axis=-1, kind="stable")[:, :, :k1]
    dist = np.take_along_axis(d2, idx, axis=-1)
    idx = idx + (np.arange(nB, dtype=np.int32) * nS)[:, None, None]
    return (idx.reshape(N, k1).astype(np.int32),
            dist.reshape(N, k1).astype(np.float32))


def kernel(coordinates, row_splits):
    coordinates = np.ascontiguousarray(coordinates, dtype=np.float32)
    rs = np.asarray(row_splits)
    expected_rs = np.arange(B + 1, dtype=np.int64) * S
    if coordinates.shape != (B * S, D) or rs.shape != (B + 1,) or \
            not np.array_equal(rs.astype(np.int64), expected_rs):
        return _numpy_fallback(coordinates, rs)

    from concourse import bass_utils

    nc = _get_nc()
    in_maps = [
        {"coords": coordinates[b * S:(b + 1) * S]} for b in range(B)
    ]
    res = bass_utils.run_bass_kernel_spmd(nc, in_maps, core_ids=list(range(B)))
    idx = np.concatenate(
        [res.results[b]["out_idx"] + np.int32(b * S) for b in range(B)], axis=0
    ).astype(np.int32)
    dist = np.concatenate(
        [res.results[b]["out_dist"] for b in range(B)], axis=0
    ).astype(np.float32)
    return idx, dist


# revision 25
# speedup vs baseline: 1.0891x; 1.0891x over previous
"""Trainium2 Bass kernel for intra-segment KNN (K=64 neighbours + self).

Problem: coordinates [32768, 4] f32 split into 8 equal segments (events) of
4096 points; per point, find the 65 nearest points (incl. self) within its
segment, returning (idx int32 [32768,65], dist f32 [32768,65]) sorted by
ascending squared distance (matching jax.lax.top_k on -d2, ties to the
lower index).

Sharding: one event per NeuronCore (8 cores), pure data parallel.

Per-core algorithm (S=4096 points, D=4 dims), "packed-key" selection:
  - negkey[r, j] = -d2[r, j] via TensorE in fp32: psum = 2*c_r.c_j - |c_j|^2
    (contraction 8: lhsT rows 0-3 = 2*c^T, rows 4-7 = -1; rhs rows 0-3 =
    c^T, rows 4-7 = (c^T)^2), then ScalarE adds the per-row bias -|c_r|^2
    while copying PSUM -> SBUF.
  - GpSimd overwrites the LOW BYTE of each fp32 negkey with (j mod 128)
    (a plain byte-strided tensor_copy from a u8 iota).  An fp32 compare of
    two such keys orders by the value's top 24 bits and, on ties, prefers
    the lower column (keys <= 0), reproducing top_k tie-breaking to 2^-16
    relative precision -- selection needs no MaxIndex8 passes at all.
  - VectorE group phase per 128-row tile: 32 groups of 128 columns, Max8
    keeps the top-8 keys of each -> C [128, 256].  (P(a 128-col group
    holds >8 of the true top-65) ~ 2e-4 -- a handful of rows per run.)
  - As the group phase streams, ScalarE incrementally extracts each new
    C chunk's local-column bytes (locb) and overwrites those bytes with
    the C-slot index q, so C is fully slot-coded moments after the last
    Max8; colf[q] = locb[q] + 128*(q>>3) = the candidate's column.
  - VectorE C phase: 9 rounds of (Max8 + MatchReplace8) over the slot-
    coded C -> top-72 keys V, sorted; each key's low byte IS its C slot.
  - Column recovery without any gather op (GpSimd double local_scatter):
    W[slot72[k]] = k+1, then Y[W[q]-1] = colf[q]; Y[:, :65] = neighbour
    columns, converted/offset on ScalarE + host.
  - dist = Relu(-V) on ScalarE (top-24-bit d2, rel err ~2^-16).

Schedule: the DVE executes in order and the C phase is a serial chain, so
emission is software-pipelined -- tile t's 32 independent group-phase Max8
ops are interleaved with tile t-1's C-phase ops to hide chain latency.
TensorE (matmuls), ScalarE (bias/byte converts), GpSimd (pack, scatters)
and DMA all run under the VectorE-bound ~12us/tile steady state.
"""

import numpy as np

S = 4096          # points per segment
D = 4             # coordinate dims
B = 8             # segments / cores
K1 = 65           # neighbours incl. self
P = 128           # partitions
NT = S // P       # 32 row tiles
GW = 128          # group width (columns per group)
NG = S // GW      # 32 groups
CW = NG * 8       # candidate array width (256)
NR = 9            # extraction rounds (9*8 = 72 >= 65)
RW = NR * 8       # 72
HB = 1024         # psum chunk width (2 banks)
NEG_BIG = -3.0e38 # "minus infinity" replacement; never equals a real key
LOCM = GW - 1     # local-column mask
SLOTM = CW - 1    # slot mask

_NC_CACHE = {}


def _build_nc():
    import concourse.bacc as bacc
    import concourse.mybir as mybir
    from concourse import bass, library_config
    from concourse.tile import TileContext

    fp32 = mybir.dt.float32
    f32r = mybir.dt.float32r
    u8 = mybir.dt.uint8
    i16 = mybir.dt.int16
    i32 = mybir.dt.int32
    Alu = mybir.AluOpType
    Act = mybir.ActivationFunctionType

    nc = bacc.Bacc(None, target_bir_lowering=False, debug=False)

    coords = nc.dram_tensor("coords", [S, D], fp32, kind="ExternalInput")
    out_dist = nc.dram_tensor("out_dist", [S, K1], fp32, kind="ExternalOutput")
    out_idx = nc.dram_tensor("out_idx", [S, K1], i32, kind="ExternalOutput")

    with TileContext(nc) as tc:
        with (
            tc.tile_pool(name="const", bufs=1) as cpool,
            tc.tile_pool(name="cand", bufs=4) as candpool,
            tc.tile_pool(name="small", bufs=6) as spool,
            tc.tile_pool(name="outs", bufs=3) as opool,
            tc.tile_pool(name="psum", bufs=3, space="PSUM") as ppool,
            tc.tile_pool(name="psumT", bufs=2, space="PSUM") as ptpool,
        ):
            # ---------------- persistent tensors ----------------
            rhs8 = cpool.tile([8, S], fp32)     # rows 0-3: c^T, rows 4-7: (c^T)^2
            lhsT8 = cpool.tile([8, S], fp32)    # rows 0-3: 2*c^T, rows 4-7: -1
            ident = cpool.tile([P, P], fp32)    # identity for PE transpose
            sqr_all = cpool.tile([P, NT], fp32) # -|c_r|^2 per row, per tile col
            nkA = cpool.tile([P, S], fp32)      # negkey/packed-key ping-pong
            nkB = cpool.tile([P, S], fp32)
            loc8 = cpool.tile([P, S], u8)       # j mod 128, as bytes
            slotb = cpool.tile([P, CW], u8)     # q (C-slot iota), as bytes
            goff = cpool.tile([P, CW], i16)     # 128 * (q >> 3)
            kio1 = cpool.tile([P, RW], i16)     # 1..72
            negone = cpool.tile([P, 1], fp32)   # -1.0 bias for rank decrement
            lscale = cpool.tile([8, 1], fp32)   # rows 0-3: 2.0, rows 4-7: 0.0
            lbias = cpool.tile([8, 1], fp32)    # rows 0-3: 0.0, rows 4-7: -1.0

            nc.vector.memset(negone, -1.0)
            # per-row scale/bias used to build lhsT8 in one activation
            nc.vector.memset(lscale, 2.0)
            nc.gpsimd.affine_select(lscale, lscale, [[0, 1]], Alu.is_ge, 0.0,
                                    base=3, channel_multiplier=-1)
            nc.vector.memset(lbias, -1.0)
            nc.gpsimd.affine_select(lbias, lbias, [[0, 1]], Alu.is_ge, 0.0,
                                    base=-4, channel_multiplier=1)

            # identity matrix: ones masked to the diagonal
            nc.vector.memset(ident, 1.0)
            nc.gpsimd.affine_select(
                ident, ident, [[1, P]], Alu.is_equal, 0.0,
                base=0, channel_multiplier=-1,
            )
            nc.gpsimd.iota(loc8, [[0, NG], [1, GW]], base=0,
                           channel_multiplier=0,
                           allow_small_or_imprecise_dtypes=True)
            nc.gpsimd.iota(slotb, [[1, CW]], base=0, channel_multiplier=0,
                           allow_small_or_imprecise_dtypes=True)
            nc.gpsimd.iota(goff, [[GW, NG], [0, 8]], base=0, channel_multiplier=0)
            nc.gpsimd.iota(kio1, [[1, RW]], base=1, channel_multiplier=0)

            # ---------------- prologue: build c^T layout ----------------
            # (engine APs must start at partition 0, so lhsT8 is written in
            # ONE activation whose per-row scale/bias produce 2*c^T on rows
            # 0-3 and -1 on rows 4-7.)
            for t in range(NT):
                ct8 = spool.tile([P, 2 * D], fp32, tag="ct8")
                # cols 0-3 <- coords rows, cols 4-7 <- squares
                nc.sync.dma_start(ct8[:, 0:D], coords[t * P:(t + 1) * P, :])
                nc.vector.tensor_tensor(out=ct8[:, D:2 * D], in0=ct8[:, 0:D],
                                        in1=ct8[:, 0:D], op=Alu.mult)
                # -|c_r|^2 for this tile's 128 rows
                nc.vector.tensor_reduce(
                    sqr_all[:, t:t + 1], ct8[:, D:2 * D],
                    axis=mybir.AxisListType.X, op=Alu.add, negate=True,
                )
                # transpose [128, 8] -> [8, 128]
                pT = ptpool.tile([2 * D, P], fp32, tag="pT")
                nc.tensor.transpose(pT, ct8, ident)
                cs = slice(t * P, (t + 1) * P)
                nc.vector.tensor_copy(rhs8[:, cs], pT)
                nc.scalar.activation(lhsT8[:, cs], pT, Act.Identity,
                                     bias=lbias, scale=lscale)

            # local_scatter lives in gpsimd ucode library 7; load it once.
            # (iota/affine_select above ran in the default library; the
            # tensor_scalar / scalar_tensor_tensor ops below are built-ins.)
            nc.gpsimd.load_library(library_config.local_scatter)

            # ---------------- main loop over row tiles ----------------
            # Software-pipelined emission: the DVE executes its queue in
            # order, and the C phase is a serial max8/match_replace chain.
            # Emitting tile t's (independent) group-phase ops interleaved
            # with tile t-1's C-phase ops lets the engine fill the chain's
            # dependency bubbles with useful work.

            def emit_cphase(st):
                """One DVE op of the pending tile's C phase; True if any."""
                r = st["round"]
                if r >= NR:
                    return False
                C2, V = st["C2"], st["V"]
                v8 = V[:, 8 * r:8 * r + 8]
                if not st["did_max"]:
                    nc.vector.max(v8, C2)
                    st["did_max"] = True
                    if r + 1 >= NR:
                        st["round"] = NR
                    return True
                nc.vector.match_replace(C2, v8, C2, NEG_BIG)
                st["did_max"] = False
                st["round"] = r + 1
                return True

            def emit_recovery(st):
                """Column unscramble + outputs for the pending tile."""
                V, colf, cs = st["V"], st["colf"], st["cs"]
                slot72 = spool.tile([P, RW], i16, tag="slot72")
                nc.scalar.activation(slot72, V.bitcast(u8)[:, 0:4 * RW:4],
                                     Act.Copy)
                W = spool.tile([P, CW], i16, tag="W")
                nc.gpsimd.local_scatter(W, kio1, slot72,
                                        channels=P, num_elems=CW, num_idxs=RW)
                Wm = spool.tile([P, CW], i16, tag="Wm")
                nc.vector.tensor_scalar_add(Wm, W, -1)
                Y = spool.tile([P, RW], i16, tag="Y")
                nc.gpsimd.local_scatter(Y, colf, Wm,
                                        channels=P, num_elems=RW, num_idxs=CW)
                dist65 = opool.tile([P, K1], fp32, tag="dist65")
                idx65 = opool.tile([P, K1], i32, tag="idx65")
                nc.scalar.activation(dist65, V[:, :K1], Act.Relu, scale=-1.0)
                nc.vector.tensor_copy(idx65, Y[:, :K1])
                nc.sync.dma_start(out_dist[cs, :], dist65)
                nc.sync.dma_start(out_idx[cs, :], idx65)

            pending = None
            for t in range(NT):
                cs = slice(t * P, (t + 1) * P)
                nk = nkA if t % 2 == 0 else nkB
                for h in range(S // HB):
                    ps = ppool.tile([P, HB], fp32, tag="ps")
                    for m in range(HB // 512):
                        col0 = h * HB + m * 512
                        nc.tensor.matmul(
                            ps[:, m * 512:(m + 1) * 512],
                            lhsT8[:, cs],
                            rhs8[:, col0:col0 + 512],
                            start=True, stop=True,
                        )
                    # negkey = psum - |c_r|^2
                    nc.scalar.activation(
                        nk[:, h * HB:(h + 1) * HB], ps,
                        Act.Identity, bias=sqr_all[:, t:t + 1],
                    )
                    # pack keys in place, chunk by chunk: the low byte of
                    # each fp32 negkey is overwritten with (j mod 128),
                    # leaving the value's top 24 bits as the sort key and
                    # the local column as tiebreaker.
                    nc.gpsimd.tensor_copy(
                        nk.bitcast(u8)[:, 4 * h * HB:4 * (h + 1) * HB:4],
                        loc8[:, h * HB:(h + 1) * HB])

                # ---- group phase (tile t) + C phase (tile t-1) ----
                # After every 4 groups, ScalarE extracts those candidates'
                # local-column bytes into locb and then overwrites the same
                # bytes with the C-slot index -- so Cv is fully slot-coded
                # (and ready for its own C phase) moments after the last
                # Max8, with no bulk repack on the critical path.
                Cv = candpool.tile([P, CW], fp32, tag="Cv")
                locb = spool.tile([P, CW], i16, tag="locb")
                colf = spool.tile([P, CW], i16, tag="colf")
                Cvb = Cv.bitcast(u8)
                for g in range(NG):
                    if pending is not None and g % 2 == 0:
                        emit_cphase(pending)
                    nc.vector.max(Cv[:, 8 * g:8 * g + 8],
                                  nk[:, g * GW:(g + 1) * GW])
                    if pending is not None and g == 1 and not pending["colf_done"]:
                        nc.vector.tensor_tensor(out=pending["colf"],
                                                in0=pending["locb"],
                                                in1=goff, op=Alu.add)
                        pending["colf_done"] = True
                    if g % 4 == 3:
                        ch = slice(8 * (g - 3), 8 * (g + 1))
                        bs = slice(32 * (g - 3), 4 * 8 * (g + 1), 4)
                        nc.scalar.activation(locb[:, ch], Cvb[:, bs],
                                             Act.Copy)
                        nc.scalar.activation(Cvb[:, bs], slotb[:, ch],
                                             Act.Copy)
                if pending is not None:
                    while emit_cphase(pending):
                        pass
                    emit_recovery(pending)

                V = spool.tile([P, RW], fp32, tag="V")
                pending = {"C2": Cv, "V": V, "locb": locb, "colf": colf,
                           "cs": cs, "round": 0, "did_max": False,
                           "colf_done": False}

            # drain the last tile's C phase + outputs
            nc.vector.tensor_tensor(out=pending["colf"], in0=pending["locb"],
                                    in1=goff, op=Alu.add)
            while emit_cphase(pending):
                pass
            emit_recovery(pending)

    nc.finalize()
    return nc


def _get_nc():
    if "nc" not in _NC_CACHE:
        _NC_CACHE["nc"] = _build_nc()
    return _NC_CACHE["nc"]


def _numpy_fallback(coordinates, row_splits):
    """Pure-numpy replica of the reference (used only on unexpected shapes)."""
    nB = int(row_splits.shape[0] - 1)
    N, nD = coordinates.shape
    nS = N // nB
    c = coordinates.reshape(nB, nS, nD).astype(np.float32)
    sq = np.sum(c * c, axis=-1)
    d2 = sq[:, :, None] + sq[:, None, :] - 2.0 * np.einsum(
        "bsd,btd->bst", c, c)
    d2 = np.maximum(d2, 0.0).astype(np.float32)
    k1 = min(K1, nS)
    idx = np.argsort(d2, axis=-1, kind="stable")[:, :, :k1]
    dist = np.take_along_axis(d2, idx, axis=-1)
    idx = idx + (np.arange(nB, dtype=np.int32) * nS)[:, None, None]
    return (idx.reshape(N, k1).astype(np.int32),
            dist.reshape(N, k1).astype(np.float32))


def kernel(coordinates, row_splits):
    coordinates = np.ascontiguousarray(coordinates, dtype=np.float32)
    rs = np.asarray(row_splits)
    expected_rs = np.arange(B + 1, dtype=np.int64) * S
    if coordinates.shape != (B * S, D) or rs.shape != (B + 1,) or \
            not np.array_equal(rs.astype(np.int64), expected_rs):
        return _numpy_fallback(coordinates, rs)

    from concourse import bass_utils

    nc = _get_nc()
    in_maps = [
        {"coords": coordinates[b * S:(b + 1) * S]} for b in range(B)
    ]
    res = bass_utils.run_bass_kernel_spmd(nc, in_maps, core_ids=list(range(B)))
    idx = np.concatenate(
        [res.results[b]["out_idx"] + np.int32(b * S) for b in range(B)], axis=0
    ).astype(np.int32)
    dist = np.concatenate(
        [res.results[b]["out_dist"] for b in range(B)], axis=0
    ).astype(np.float32)
    return idx, dist


# revision 32
# speedup vs baseline: 1.1156x; 1.0243x over previous
"""Trainium2 Bass kernel for intra-segment KNN (K=64 neighbours + self).

Problem: coordinates [32768, 4] f32 split into 8 equal segments (events) of
4096 points; per point, find the 65 nearest points (incl. self) within its
segment, returning (idx int32 [32768,65], dist f32 [32768,65]) sorted by
ascending squared distance (matching jax.lax.top_k on -d2, ties to the
lower index).

Sharding: one event per NeuronCore (8 cores), pure data parallel.

Per-core algorithm (S=4096 points, D=4 dims), "packed-key" selection:
  - negkey[r, j] = -d2[r, j] via TensorE in fp32: psum = 2*c_r.c_j - |c_j|^2
    (contraction 8: lhsT rows 0-3 = 2*c^T, rows 4-7 = -1; rhs rows 0-3 =
    c^T, rows 4-7 = (c^T)^2), then ScalarE adds the per-row bias -|c_r|^2
    while copying PSUM -> SBUF.
  - GpSimd overwrites the LOW BYTE of each fp32 negkey with (j mod 128)
    (a plain byte-strided tensor_copy from a u8 iota).  An fp32 compare of
    two such keys orders by the value's top 24 bits and, on ties, prefers
    the lower column (keys <= 0), reproducing top_k tie-breaking to 2^-16
    relative precision -- selection needs no MaxIndex8 passes at all.
  - VectorE group phase per 128-row tile: 32 groups of 128 columns, Max8
    keeps the top-8 keys of each -> C [128, 256].  (P(a 128-col group
    holds >8 of the true top-65) ~ 2e-4 -- a handful of rows per run.)
  - As the group phase streams, ScalarE incrementally extracts each new
    C chunk's local-column bytes (locb) and overwrites those bytes with
    the C-slot index q, so C is fully slot-coded moments after the last
    Max8; colf[q] = locb[q] + 128*(q>>3) = the candidate's column.
  - VectorE C phase: 9 rounds of (Max8 + MatchReplace8) over the slot-
    coded C -> top-72 keys V, sorted; each key's low byte IS its C slot.
  - Column recovery without any gather op (GpSimd double local_scatter):
    W[slot72[k]] = k+1, then Y[W[q]-1] = colf[q]; Y[:, :65] = neighbour
    columns, converted/offset on ScalarE + host.
  - dist = Relu(-V) on ScalarE (top-24-bit d2, rel err ~2^-16).

Schedule: the DVE executes in order and the C phase is a serial chain, so
emission is software-pipelined -- tile t's 32 independent group-phase Max8
ops are interleaved with tile t-1's C-phase ops to hide chain latency.
TensorE (matmuls), ScalarE (bias/byte converts), GpSimd (pack, scatters)
and DMA all run under the VectorE-bound ~12us/tile steady state.
"""

import numpy as np

S = 4096          # points per segment
D = 4             # coordinate dims
B = 8             # segments / cores
K1 = 65           # neighbours incl. self
P = 128           # partitions
NT = S // P       # 32 row tiles
GW = 128          # group width (columns per group)
NG = S // GW      # 32 groups
CW = NG * 8       # candidate array width (256)
NR = 9            # extraction rounds (9*8 = 72 >= 65)
RW = NR * 8       # 72
HB = 512          # psum chunk width (1 bank)
NEG_BIG = -3.0e38 # "minus infinity" replacement; never equals a real key
LOCM = GW - 1     # local-column mask
SLOTM = CW - 1    # slot mask

_NC_CACHE = {}


def _build_nc():
    import concourse.bacc as bacc
    import concourse.mybir as mybir
    from concourse import bass, library_config
    from concourse.tile import TileContext

    fp32 = mybir.dt.float32
    f32r = mybir.dt.float32r
    u8 = mybir.dt.uint8
    i16 = mybir.dt.int16
    i32 = mybir.dt.int32
    Alu = mybir.AluOpType
    Act = mybir.ActivationFunctionType

    nc = bacc.Bacc(None, target_bir_lowering=False, debug=False)

    coords = nc.dram_tensor("coords", [S, D], fp32, kind="ExternalInput")
    out_dist = nc.dram_tensor("out_dist", [S, K1], fp32, kind="ExternalOutput")
    out_idx = nc.dram_tensor("out_idx", [S, K1], i32, kind="ExternalOutput")

    with TileContext(nc) as tc:
        with (
            tc.tile_pool(name="const", bufs=1) as cpool,
            tc.tile_pool(name="cand", bufs=4) as candpool,
            tc.tile_pool(name="small", bufs=6) as spool,
            tc.tile_pool(name="outs", bufs=3) as opool,
            tc.tile_pool(name="psum", bufs=4, space="PSUM") as ppool,
            tc.tile_pool(name="psumT", bufs=4, space="PSUM") as ptpool,
        ):
            # ---------------- persistent tensors ----------------
            rhs8 = cpool.tile([8, S], fp32)     # rows 0-3: c^T, rows 4-7: (c^T)^2
            lhsT8 = cpool.tile([8, S], fp32)    # rows 0-3: 2*c^T, rows 4-7: -1
            ident = cpool.tile([P, P], fp32)    # identity for PE transpose
            sqr_all = cpool.tile([P, NT], fp32) # -|c_r|^2 per row, per tile col
            nkA = cpool.tile([P, S], fp32)      # negkey/packed-key ping-pong
            nkB = cpool.tile([P, S], fp32)
            loc8 = cpool.tile([P, S], u8)       # j mod 128, as bytes
            slotb = cpool.tile([P, CW], u8)     # q (C-slot iota), as bytes
            goff = cpool.tile([P, CW], i16)     # 128 * (q >> 3)
            kio1 = cpool.tile([P, RW], i16)     # 1..72
            negone = cpool.tile([P, 1], fp32)   # -1.0 bias for rank decrement
            lscale = cpool.tile([8, 1], fp32)   # rows 0-3: 2.0, rows 4-7: 0.0
            lbias = cpool.tile([8, 1], fp32)    # rows 0-3: 0.0, rows 4-7: -1.0

            nc.vector.memset(negone, -1.0)
            # per-row scale/bias used to build lhsT8 in one activation
            nc.vector.memset(lscale, 2.0)
            nc.gpsimd.affine_select(lscale, lscale, [[0, 1]], Alu.is_ge, 0.0,
                                    base=3, channel_multiplier=-1)
            nc.vector.memset(lbias, -1.0)
            nc.gpsimd.affine_select(lbias, lbias, [[0, 1]], Alu.is_ge, 0.0,
                                    base=-4, channel_multiplier=1)

            # identity matrix: ones masked to the diagonal
            nc.vector.memset(ident, 1.0)
            nc.gpsimd.affine_select(
                ident, ident, [[1, P]], Alu.is_equal, 0.0,
                base=0, channel_multiplier=-1,
            )
            nc.gpsimd.iota(loc8, [[0, NG], [1, GW]], base=0,
                           channel_multiplier=0,
                           allow_small_or_imprecise_dtypes=True)
            nc.gpsimd.iota(slotb, [[1, CW]], base=0, channel_multiplier=0,
                           allow_small_or_imprecise_dtypes=True)
            nc.gpsimd.iota(goff, [[GW, NG], [0, 8]], base=0, channel_multiplier=0)
            nc.gpsimd.iota(kio1, [[1, RW]], base=1, channel_multiplier=0)

            # ---------------- prologue: build c^T layout ----------------
            # (engine APs must start at partition 0, so lhsT8 is written in
            # ONE activation whose per-row scale/bias produce 2*c^T on rows
            # 0-3 and -1 on rows 4-7.)
            for t4 in range(NT // 4):
                # one [8, 512] PSUM tile (a single bank) collects four
                # tiles' transposes so the rhs8/lhsT8 writes batch 4x.
                pT4 = ptpool.tile([2 * D, 4 * P], fp32, tag="pT4")
                for k in range(4):
                    t = 4 * t4 + k
                    ct8 = spool.tile([P, 2 * D], fp32, tag="ct8")
                    # cols 0-3 <- coords rows, cols 4-7 <- squares
                    nc.sync.dma_start(ct8[:, 0:D],
                                      coords[t * P:(t + 1) * P, :])
                    nc.vector.tensor_tensor(out=ct8[:, D:2 * D],
                                            in0=ct8[:, 0:D],
                                            in1=ct8[:, 0:D], op=Alu.mult)
                    # -|c_r|^2 for this tile's 128 rows
                    nc.vector.tensor_reduce(
                        sqr_all[:, t:t + 1], ct8[:, D:2 * D],
                        axis=mybir.AxisListType.X, op=Alu.add, negate=True,
                    )
                    # transpose [128, 8] -> [8, 128]
                    nc.tensor.transpose(pT4[:, k * P:(k + 1) * P], ct8, ident)
                cs4 = slice(4 * t4 * P, (4 * t4 + 4) * P)
                nc.vector.tensor_copy(rhs8[:, cs4], pT4)
                nc.scalar.activation(lhsT8[:, cs4], pT4, Act.Identity,
                                     bias=lbias, scale=lscale)

            # local_scatter lives in gpsimd ucode library 7; load it once.
            # (iota/affine_select above ran in the default library; the
            # tensor_scalar / scalar_tensor_tensor ops below are built-ins.)
            nc.gpsimd.load_library(library_config.local_scatter)

            # ---------------- main loop over row tiles ----------------
            # Software-pipelined emission: the DVE executes its queue in
            # order, and the C phase is a serial max8/match_replace chain.
            # Emitting tile t's (independent) group-phase ops interleaved
            # with tile t-1's C-phase ops lets the engine fill the chain's
            # dependency bubbles with useful work.

            def emit_cphase(st):
                """One DVE op of the pending tile's C phase; True if any."""
                r = st["round"]
                if r >= NR:
                    return False
                C2, V = st["C2"], st["V"]
                v8 = V[:, 8 * r:8 * r + 8]
                if not st["did_max"]:
                    nc.vector.max(v8, C2)
                    st["did_max"] = True
                    if r + 1 >= NR:
                        st["round"] = NR
                    return True
                nc.vector.match_replace(C2, v8, C2, NEG_BIG)
                st["did_max"] = False
                st["round"] = r + 1
                return True

            def emit_recovery(st):
                """Column unscramble + outputs for the pending tile."""
                V, colf, cs = st["V"], st["colf"], st["cs"]
                slot72 = spool.tile([P, RW], i16, tag="slot72")
                nc.scalar.activation(slot72, V.bitcast(u8)[:, 0:4 * RW:4],
                                     Act.Copy)
                W = spool.tile([P, CW], i16, tag="W")
                nc.gpsimd.local_scatter(W, kio1, slot72,
                                        channels=P, num_elems=CW, num_idxs=RW)
                Wm = spool.tile([P, CW], i16, tag="Wm")
                nc.vector.tensor_scalar_add(Wm, W, -1)
                Y = spool.tile([P, RW], i16, tag="Y")
                nc.gpsimd.local_scatter(Y, colf, Wm,
                                        channels=P, num_elems=RW, num_idxs=CW)
                dist65 = opool.tile([P, K1], fp32, tag="dist65")
                idx65 = opool.tile([P, K1], i32, tag="idx65")
                nc.scalar.activation(dist65, V[:, :K1], Act.Relu, scale=-1.0)
                nc.vector.tensor_copy(idx65, Y[:, :K1])
                nc.sync.dma_start(out_dist[cs, :], dist65)
                nc.sync.dma_start(out_idx[cs, :], idx65)

            pending = None
            for t in range(NT):
                cs = slice(t * P, (t + 1) * P)
                nk = nkA if t % 2 == 0 else nkB
                for h in range(S // HB):
                    ps = ppool.tile([P, HB], fp32, tag="ps")
                    for m in range(HB // 512):
                        col0 = h * HB + m * 512
                        nc.tensor.matmul(
                            ps[:, m * 512:(m + 1) * 512],
                            lhsT8[:, cs],
                            rhs8[:, col0:col0 + 512],
                            start=True, stop=True,
                        )
                    # negkey = psum - |c_r|^2
                    nc.scalar.activation(
                        nk[:, h * HB:(h + 1) * HB], ps,
                        Act.Identity, bias=sqr_all[:, t:t + 1],
                    )
                    # pack keys in place, chunk by chunk: the low byte of
                    # each fp32 negkey is overwritten with (j mod 128),
                    # leaving the value's top 24 bits as the sort key and
                    # the local column as tiebreaker.
                    for qq in range(2):
                        c0 = h * HB + qq * (HB // 2)
                        c1 = c0 + HB // 2
                        nc.gpsimd.tensor_copy(
                            nk.bitcast(u8)[:, 4 * c0:4 * c1:4],
                            loc8[:, c0:c1])

                # ---- group phase (tile t) + C phase (tile t-1) ----
                # After every 4 groups, ScalarE extracts those candidates'
                # local-column bytes into locb and then overwrites the same
                # bytes with the C-slot index -- so Cv is fully slot-coded
                # (and ready for its own C phase) moments after the last
                # Max8, with no bulk repack on the critical path.
                Cv = candpool.tile([P, CW], fp32, tag="Cv")
                locb = spool.tile([P, CW], i16, tag="locb")
                colf = spool.tile([P, CW], i16, tag="colf")
                Cvb = Cv.bitcast(u8)
                for g in range(NG):
                    pass
                    nc.vector.max(Cv[:, 8 * g:8 * g + 8],
                                  nk[:, g * GW:(g + 1) * GW])
                    if pending is not None and g % 2 == 1:
                        emit_cphase(pending)
                    if pending is not None and g == 1 and not pending["colf_done"]:
                        nc.vector.tensor_tensor(out=pending["colf"],
                                                in0=pending["locb"],
                                                in1=goff, op=Alu.add)
                        pending["colf_done"] = True
                    if g % 4 == 3:
                        ch = slice(8 * (g - 3), 8 * (g + 1))
                        bs = slice(32 * (g - 3), 4 * 8 * (g + 1), 4)
                        nc.scalar.activation(locb[:, ch], Cvb[:, bs],
                                             Act.Copy)
                        nc.scalar.activation(Cvb[:, bs], slotb[:, ch],
                                             Act.Copy)
                if pending is not None:
                    while emit_cphase(pending):
                        pass
                    emit_recovery(pending)

                V = spool.tile([P, RW], fp32, tag="V")
                pending = {"C2": Cv, "V": V, "locb": locb, "colf": colf,
                           "cs": cs, "round": 0, "did_max": False,
                           "colf_done": False}

            # drain the last tile's C phase + outputs
            nc.vector.tensor_tensor(out=pending["colf"], in0=pending["locb"],
                                    in1=goff, op=Alu.add)
            while emit_cphase(pending):
                pass
            emit_recovery(pending)

    nc.finalize()
    return nc


def _get_nc():
    if "nc" not in _NC_CACHE:
        _NC_CACHE["nc"] = _build_nc()
    return _NC_CACHE["nc"]


def _numpy_fallback(coordinates, row_splits):
    """Pure-numpy replica of the reference (used only on unexpected shapes)."""
    nB = int(row_splits.shape[0] - 1)
    N, nD = coordinates.shape
    nS = N // nB
    c = coordinates.reshape(nB, nS, nD).astype(np.float32)
    sq = np.sum(c * c, axis=-1)
    d2 = sq[:, :, None] + sq[:, None, :] - 2.0 * np.einsum(
        "bsd,btd->bst", c, c)
    d2 = np.maximum(d2, 0.0).astype(np.float32)
    k1 = min(K1, nS)
    idx = np.argsort(d2, axis=-1, kind="stable")[:, :, :k1]
    dist = np.take_along_axis(d2, idx, axis=-1)
    idx = idx + (np.arange(nB, dtype=np.int32) * nS)[:, None, None]
    return (idx.reshape(N, k1).astype(np.int32),
            dist.reshape(N, k1).astype(np.float32))


def kernel(coordinates, row_splits):
    coordinates = np.ascontiguousarray(coordinates, dtype=np.float32)
    rs = np.asarray(row_splits)
    expected_rs = np.arange(B + 1, dtype=np.int64) * S
    if coordinates.shape != (B * S, D) or rs.shape != (B + 1,) or \
            not np.array_equal(rs.astype(np.int64), expected_rs):
        return _numpy_fallback(coordinates, rs)

    from concourse import bass_utils

    nc = _get_nc()
    in_maps = [
        {"coords": coordinates[b * S:(b + 1) * S]} for b in range(B)
    ]
    res = bass_utils.run_bass_kernel_spmd(nc, in_maps, core_ids=list(range(B)))
    idx = np.concatenate(
        [res.results[b]["out_idx"] + np.int32(b * S) for b in range(B)], axis=0
    ).astype(np.int32)
    dist = np.concatenate(
        [res.results[b]["out_dist"] for b in range(B)], axis=0
    ).astype(np.float32)
    return idx, dist


# revision 38
# speedup vs baseline: 1.1215x; 1.0054x over previous
"""Trainium2 Bass kernel for intra-segment KNN (K=64 neighbours + self).

Problem: coordinates [32768, 4] f32 split into 8 equal segments (events) of
4096 points; per point, find the 65 nearest points (incl. self) within its
segment, returning (idx int32 [32768,65], dist f32 [32768,65]) sorted by
ascending squared distance (matching jax.lax.top_k on -d2, ties to the
lower index).

Sharding: one event per NeuronCore (8 cores), pure data parallel.

Per-core algorithm (S=4096 points, D=4 dims), "packed-key" selection:
  - negkey[r, j] = -d2[r, j] via TensorE in fp32: psum = 2*c_r.c_j - |c_j|^2
    (contraction 8: lhsT rows 0-3 = 2*c^T, rows 4-7 = -1; rhs rows 0-3 =
    c^T, rows 4-7 = (c^T)^2), then ScalarE adds the per-row bias -|c_r|^2
    while copying PSUM -> SBUF.
  - GpSimd overwrites the LOW BYTE of each fp32 negkey with (j mod 128)
    (a plain byte-strided tensor_copy from a u8 iota).  An fp32 compare of
    two such keys orders by the value's top 24 bits and, on ties, prefers
    the lower column (keys <= 0), reproducing top_k tie-breaking to 2^-16
    relative precision -- selection needs no MaxIndex8 passes at all.
  - VectorE group phase per 128-row tile: 32 groups of 128 columns, Max8
    keeps the top-8 keys of each -> C [128, 256].  (P(a 128-col group
    holds >8 of the true top-65) ~ 2e-4 -- a handful of rows per run.)
  - As the group phase streams, ScalarE incrementally extracts each new
    C chunk's local-column bytes (locb) and overwrites those bytes with
    the C-slot index q, so C is fully slot-coded moments after the last
    Max8; colf[q] = locb[q] + 128*(q>>3) = the candidate's column.
  - VectorE C phase: 9 rounds of (Max8 + MatchReplace8) over the slot-
    coded C -> top-72 keys V, sorted; each key's low byte IS its C slot.
  - Column recovery without any gather op (GpSimd double local_scatter):
    W[slot72[k]] = k+1, then Y[W[q]-1] = colf[q]; Y[:, :65] = neighbour
    columns, converted/offset on ScalarE + host.
  - dist = Relu(-V) on ScalarE (top-24-bit d2, rel err ~2^-16).

Schedule: the DVE executes in order and the C phase is a serial chain, so
emission is software-pipelined -- tile t's 32 independent group-phase Max8
ops are interleaved with tile t-1's C-phase ops to hide chain latency.
TensorE (matmuls), ScalarE (bias/byte converts), GpSimd (pack, scatters)
and DMA all run under the VectorE-bound ~12us/tile steady state.
"""

import numpy as np

S = 4096          # points per segment
D = 4             # coordinate dims
B = 8             # segments / cores
K1 = 65           # neighbours incl. self
P = 128           # partitions
NT = S // P       # 32 row tiles
GW = 128          # group width (columns per group)
NG = S // GW      # 32 groups
CW = NG * 8       # candidate array width (256)
NR = 9            # extraction rounds (9*8 = 72 >= 65)
RW = NR * 8       # 72
HB = 512          # psum chunk width (1 bank)
NEG_BIG = -3.0e38 # "minus infinity" replacement; never equals a real key
LOCM = GW - 1     # local-column mask
SLOTM = CW - 1    # slot mask

_NC_CACHE = {}


def _build_nc():
    import concourse.bacc as bacc
    import concourse.mybir as mybir
    from concourse import bass, library_config
    from concourse.tile import TileContext

    fp32 = mybir.dt.float32
    f32r = mybir.dt.float32r
    u8 = mybir.dt.uint8
    i16 = mybir.dt.int16
    i32 = mybir.dt.int32
    Alu = mybir.AluOpType
    Act = mybir.ActivationFunctionType

    nc = bacc.Bacc(None, target_bir_lowering=False, debug=False)

    coords = nc.dram_tensor("coords", [S, D], fp32, kind="ExternalInput")
    out_dist = nc.dram_tensor("out_dist", [S, K1], fp32, kind="ExternalOutput")
    out_idx = nc.dram_tensor("out_idx", [S, K1], i32, kind="ExternalOutput")

    with TileContext(nc) as tc:
        with (
            tc.tile_pool(name="const", bufs=1) as cpool,
            tc.tile_pool(name="cand", bufs=4) as candpool,
            tc.tile_pool(name="small", bufs=8) as spool,
            tc.tile_pool(name="outs", bufs=3) as opool,
            tc.tile_pool(name="psum", bufs=4, space="PSUM") as ppool,
            tc.tile_pool(name="psumT", bufs=4, space="PSUM") as ptpool,
        ):
            # ---------------- persistent tensors ----------------
            rhs8 = cpool.tile([8, S], fp32)     # rows 0-3: c^T, rows 4-7: (c^T)^2
            lhsT8 = cpool.tile([8, S], fp32)    # rows 0-3: 2*c^T, rows 4-7: -1
            ident = cpool.tile([P, P], fp32)    # identity for PE transpose
            sqr_all = cpool.tile([P, NT], fp32) # -|c_r|^2 per row, per tile col
            nkA = cpool.tile([P, S], fp32)      # negkey/packed-key ping-pong
            nkB = cpool.tile([P, S], fp32)
            loc8 = cpool.tile([P, S], u8)       # j mod 128, as bytes
            slotb = cpool.tile([P, CW], u8)     # q (C-slot iota), as bytes
            goff = cpool.tile([P, CW], i16)     # 128 * (q >> 3)
            kio1 = cpool.tile([P, RW], i16)     # 1..72
            negone = cpool.tile([P, 1], fp32)   # -1.0 bias for rank decrement
            lscale = cpool.tile([8, 1], fp32)   # rows 0-3: 2.0, rows 4-7: 0.0
            lbias = cpool.tile([8, 1], fp32)    # rows 0-3: 0.0, rows 4-7: -1.0

            nc.vector.memset(negone, -1.0)
            # per-row scale/bias used to build lhsT8 in one activation
            nc.vector.memset(lscale, 2.0)
            nc.gpsimd.affine_select(lscale, lscale, [[0, 1]], Alu.is_ge, 0.0,
                                    base=3, channel_multiplier=-1)
            nc.vector.memset(lbias, -1.0)
            nc.gpsimd.affine_select(lbias, lbias, [[0, 1]], Alu.is_ge, 0.0,
                                    base=-4, channel_multiplier=1)

            # identity matrix: ones masked to the diagonal
            nc.vector.memset(ident, 1.0)
            nc.gpsimd.affine_select(
                ident, ident, [[1, P]], Alu.is_equal, 0.0,
                base=0, channel_multiplier=-1,
            )
            nc.gpsimd.iota(loc8, [[0, NG], [1, GW]], base=0,
                           channel_multiplier=0,
                           allow_small_or_imprecise_dtypes=True)
            nc.gpsimd.iota(slotb, [[1, CW]], base=0, channel_multiplier=0,
                           allow_small_or_imprecise_dtypes=True)
            nc.gpsimd.iota(goff, [[GW, NG], [0, 8]], base=0, channel_multiplier=0)
            nc.gpsimd.iota(kio1, [[1, RW]], base=1, channel_multiplier=0)

            # ---------------- prologue: build c^T layout ----------------
            # (engine APs must start at partition 0, so lhsT8 is written in
            # ONE activation whose per-row scale/bias produce 2*c^T on rows
            # 0-3 and -1 on rows 4-7.)
            for t4 in range(NT // 4):
                # one [8, 512] PSUM tile (a single bank) collects four
                # tiles' transposes so the rhs8/lhsT8 writes batch 4x.
                pT4 = ptpool.tile([2 * D, 4 * P], fp32, tag="pT4")
                ct32 = spool.tile([P, 8 * D], fp32, tag="ct32")
                cview = ct32.rearrange("p (k two d) -> p k two d", k=4, two=2)
                for k in range(4):
                    t = 4 * t4 + k
                    # cols 8k..8k+3 <- coords rows; 8k+4..8k+7 <- squares
                    nc.sync.dma_start(ct32[:, 8 * k:8 * k + D],
                                      coords[t * P:(t + 1) * P, :])
                nc.vector.tensor_tensor(out=cview[:, :, 1, :],
                                        in0=cview[:, :, 0, :],
                                        in1=cview[:, :, 0, :], op=Alu.mult)
                # -|c_r|^2 for these four tiles' 4x128 rows
                nc.vector.tensor_reduce(
                    sqr_all[:, 4 * t4:4 * t4 + 4], cview[:, :, 1, :],
                    axis=mybir.AxisListType.X, op=Alu.add, negate=True,
                )
                for k in range(4):
                    # transpose [128, 8] -> [8, 128]
                    nc.tensor.transpose(pT4[:, k * P:(k + 1) * P],
                                        ct32[:, 8 * k:8 * k + 2 * D], ident)
                cs4 = slice(4 * t4 * P, (4 * t4 + 4) * P)
                nc.vector.tensor_copy(rhs8[:, cs4], pT4)
                nc.scalar.activation(lhsT8[:, cs4], pT4, Act.Identity,
                                     bias=lbias, scale=lscale)

            # local_scatter lives in gpsimd ucode library 7; load it once.
            # (iota/affine_select above ran in the default library; the
            # tensor_scalar / scalar_tensor_tensor ops below are built-ins.)
            nc.gpsimd.load_library(library_config.local_scatter)

            # ---------------- main loop over row tiles ----------------
            # Software-pipelined emission: the DVE executes its queue in
            # order, and the C phase is a serial max8/match_replace chain.
            # Emitting tile t's (independent) group-phase ops interleaved
            # with tile t-1's C-phase ops lets the engine fill the chain's
            # dependency bubbles with useful work.

            def emit_cphase(st):
                """One DVE op of the pending tile's C phase; True if any."""
                r = st["round"]
                if r >= NR:
                    return False
                C2, V = st["C2"], st["V"]
                v8 = V[:, 8 * r:8 * r + 8]
                if not st["did_max"]:
                    nc.vector.max(v8, C2)
                    st["did_max"] = True
                    if r + 1 >= NR:
                        st["round"] = NR
                    return True
                nc.vector.match_replace(C2, v8, C2, NEG_BIG)
                st["did_max"] = False
                st["round"] = r + 1
                return True

            def emit_recovery(st):
                """Column unscramble + outputs for the pending tile."""
                V, colf, cs = st["V"], st["colf"], st["cs"]
                slot72 = spool.tile([P, RW], i16, tag="slot72")
                nc.scalar.activation(slot72, V.bitcast(u8)[:, 0:4 * RW:4],
                                     Act.Copy)
                W = spool.tile([P, CW], i16, tag="W")
                nc.gpsimd.local_scatter(W, kio1, slot72,
                                        channels=P, num_elems=CW, num_idxs=RW)
                Wm = spool.tile([P, CW], i16, tag="Wm")
                nc.vector.tensor_scalar_add(Wm, W, -1)
                Y = spool.tile([P, RW], i16, tag="Y")
                nc.gpsimd.local_scatter(Y, colf, Wm,
                                        channels=P, num_elems=RW, num_idxs=CW)
                dist65 = opool.tile([P, K1], fp32, tag="dist65")
                idx65 = opool.tile([P, K1], i32, tag="idx65")
                nc.scalar.activation(dist65, V[:, :K1], Act.Relu, scale=-1.0)
                nc.vector.tensor_copy(idx65, Y[:, :K1])
                nc.sync.dma_start(out_dist[cs, :], dist65)
                nc.sync.dma_start(out_idx[cs, :], idx65)

            pending = None
            for t in range(NT):
                cs = slice(t * P, (t + 1) * P)
                nk = nkA if t % 2 == 0 else nkB
                for h in range(S // HB):
                    ps = ppool.tile([P, HB], fp32, tag="ps")
                    for m in range(HB // 512):
                        col0 = h * HB + m * 512
                        nc.tensor.matmul(
                            ps[:, m * 512:(m + 1) * 512],
                            lhsT8[:, cs],
                            rhs8[:, col0:col0 + 512],
                            start=True, stop=True,
                        )
                    # negkey = psum - |c_r|^2
                    nc.scalar.activation(
                        nk[:, h * HB:(h + 1) * HB], ps,
                        Act.Identity, bias=sqr_all[:, t:t + 1],
                    )
                    # pack keys in place, chunk by chunk: the low byte of
                    # each fp32 negkey is overwritten with (j mod 128),
                    # leaving the value's top 24 bits as the sort key and
                    # the local column as tiebreaker.
                    for qq in range(2):
                        c0 = h * HB + qq * (HB // 2)
                        c1 = c0 + HB // 2
                        nc.gpsimd.tensor_copy(
                            nk.bitcast(u8)[:, 4 * c0:4 * c1:4],
                            loc8[:, c0:c1])

                # ---- group phase (tile t) + C phase (tile t-1) ----
                # After every 4 groups, ScalarE extracts those candidates'
                # local-column bytes into locb and then overwrites the same
                # bytes with the C-slot index -- so Cv is fully slot-coded
                # (and ready for its own C phase) moments after the last
                # Max8, with no bulk repack on the critical path.
                Cv = candpool.tile([P, CW], fp32, tag="Cv")
                locb = spool.tile([P, CW], i16, tag="locb")
                colf = spool.tile([P, CW], i16, tag="colf")
                Cvb = Cv.bitcast(u8)
                for g in range(NG):
                    pass
                    nc.vector.max(Cv[:, 8 * g:8 * g + 8],
                                  nk[:, g * GW:(g + 1) * GW])
                    if pending is not None and g % 2 == 1:
                        emit_cphase(pending)
                    if pending is not None and g == 1 and not pending["colf_done"]:
                        nc.vector.tensor_tensor(out=pending["colf"],
                                                in0=pending["locb"],
                                                in1=goff, op=Alu.add)
                        pending["colf_done"] = True
                    if g % 4 == 3:
                        ch = slice(8 * (g - 3), 8 * (g + 1))
                        bs = slice(32 * (g - 3), 4 * 8 * (g + 1), 4)
                        nc.scalar.activation(locb[:, ch], Cvb[:, bs],
                                             Act.Copy)
                        nc.scalar.activation(Cvb[:, bs], slotb[:, ch],
                                             Act.Copy)
                if pending is not None:
                    while emit_cphase(pending):
                        pass
                    emit_recovery(pending)

                V = spool.tile([P, RW], fp32, tag="V")
                pending = {"C2": Cv, "V": V, "locb": locb, "colf": colf,
                           "cs": cs, "round": 0, "did_max": False,
                           "colf_done": False}

            # drain the last tile's C phase + outputs
            nc.vector.tensor_tensor(out=pending["colf"], in0=pending["locb"],
                                    in1=goff, op=Alu.add)
            while emit_cphase(pending):
                pass
            emit_recovery(pending)

    nc.finalize()
    return nc


def _get_nc():
    if "nc" not in _NC_CACHE:
        _NC_CACHE["nc"] = _build_nc()
    return _NC_CACHE["nc"]


def _numpy_fallback(coordinates, row_splits):
    """Pure-numpy replica of the reference (used only on unexpected shapes)."""
    nB = int(row_splits.shape[0] - 1)
    N, nD = coordinates.shape
    nS = N // nB
    c = coordinates.reshape(nB, nS, nD).astype(np.float32)
    sq = np.sum(c * c, axis=-1)
    d2 = sq[:, :, None] + sq[:, None, :] - 2.0 * np.einsum(
        "bsd,btd->bst", c, c)
    d2 = np.maximum(d2, 0.0).astype(np.float32)
    k1 = min(K1, nS)
    idx = np.argsort(d2, axis=-1, kind="stable")[:, :, :k1]
    dist = np.take_along_axis(d2, idx, axis=-1)
    idx = idx + (np.arange(nB, dtype=np.int32) * nS)[:, None, None]
    return (idx.reshape(N, k1).astype(np.int32),
            dist.reshape(N, k1).astype(np.float32))


def kernel(coordinates, row_splits):
    coordinates = np.ascontiguousarray(coordinates, dtype=np.float32)
    rs = np.asarray(row_splits)
    expected_rs = np.arange(B + 1, dtype=np.int64) * S
    if coordinates.shape != (B * S, D) or rs.shape != (B + 1,) or \
            not np.array_equal(rs.astype(np.int64), expected_rs):
        return _numpy_fallback(coordinates, rs)

    from concourse import bass_utils

    nc = _get_nc()
    in_maps = [
        {"coords": coordinates[b * S:(b + 1) * S]} for b in range(B)
    ]
    res = bass_utils.run_bass_kernel_spmd(nc, in_maps, core_ids=list(range(B)))
    idx = np.concatenate(
        [res.results[b]["out_idx"] + np.int32(b * S) for b in range(B)], axis=0
    ).astype(np.int32)
    dist = np.concatenate(
        [res.results[b]["out_dist"] for b in range(B)], axis=0
    ).astype(np.float32)
    return idx, dist


# revision 39
# speedup vs baseline: 1.1233x; 1.0016x over previous
"""Trainium2 Bass kernel for intra-segment KNN (K=64 neighbours + self).

Problem: coordinates [32768, 4] f32 split into 8 equal segments (events) of
4096 points; per point, find the 65 nearest points (incl. self) within its
segment, returning (idx int32 [32768,65], dist f32 [32768,65]) sorted by
ascending squared distance (matching jax.lax.top_k on -d2, ties to the
lower index).

Sharding: one event per NeuronCore (8 cores), pure data parallel.

Per-core algorithm (S=4096 points, D=4 dims), "packed-key" selection:
  - negkey[r, j] = -d2[r, j] via TensorE in fp32: psum = 2*c_r.c_j - |c_j|^2
    (contraction 8: lhsT rows 0-3 = 2*c^T, rows 4-7 = -1; rhs rows 0-3 =
    c^T, rows 4-7 = (c^T)^2), then ScalarE adds the per-row bias -|c_r|^2
    while copying PSUM -> SBUF.
  - GpSimd overwrites the LOW BYTE of each fp32 negkey with (j mod 128)
    (a plain byte-strided tensor_copy from a u8 iota).  An fp32 compare of
    two such keys orders by the value's top 24 bits and, on ties, prefers
    the lower column (keys <= 0), reproducing top_k tie-breaking to 2^-16
    relative precision -- selection needs no MaxIndex8 passes at all.
  - VectorE group phase per 128-row tile: 32 groups of 128 columns, Max8
    keeps the top-8 keys of each -> C [128, 256].  (P(a 128-col group
    holds >8 of the true top-65) ~ 2e-4 -- a handful of rows per run.)
  - As the group phase streams, ScalarE incrementally extracts each new
    C chunk's local-column bytes (locb) and overwrites those bytes with
    the C-slot index q, so C is fully slot-coded moments after the last
    Max8; colf[q] = locb[q] + 128*(q>>3) = the candidate's column.
  - VectorE C phase: 9 rounds of (Max8 + MatchReplace8) over the slot-
    coded C -> top-72 keys V, sorted; each key's low byte IS its C slot.
  - Column recovery without any gather op (GpSimd double local_scatter):
    W[slot72[k]] = k+1, then Y[W[q]-1] = colf[q]; Y[:, :65] = neighbour
    columns, converted/offset on ScalarE + host.
  - dist = Relu(-V) on ScalarE (top-24-bit d2, rel err ~2^-16).

Schedule: the DVE executes in order and the C phase is a serial chain, so
emission is software-pipelined -- tile t's 32 independent group-phase Max8
ops are interleaved with tile t-1's C-phase ops to hide chain latency.
TensorE (matmuls), ScalarE (bias/byte converts), GpSimd (pack, scatters)
and DMA all run under the VectorE-bound ~12us/tile steady state.
"""

import numpy as np

S = 4096          # points per segment
D = 4             # coordinate dims
B = 8             # segments / cores
K1 = 65           # neighbours incl. self
P = 128           # partitions
NT = S // P       # 32 row tiles
GW = 128          # group width (columns per group)
NG = S // GW      # 32 groups
CW = NG * 8       # candidate array width (256)
NR = 9            # extraction rounds (9*8 = 72 >= 65)
RW = NR * 8       # 72
HB = 512          # psum chunk width (1 bank)
NEG_BIG = -3.0e38 # "minus infinity" replacement; never equals a real key
LOCM = GW - 1     # local-column mask
SLOTM = CW - 1    # slot mask

_NC_CACHE = {}


def _build_nc():
    import concourse.bacc as bacc
    import concourse.mybir as mybir
    from concourse import bass, library_config
    from concourse.tile import TileContext

    fp32 = mybir.dt.float32
    f32r = mybir.dt.float32r
    u8 = mybir.dt.uint8
    i16 = mybir.dt.int16
    i32 = mybir.dt.int32
    Alu = mybir.AluOpType
    Act = mybir.ActivationFunctionType

    nc = bacc.Bacc(None, target_bir_lowering=False, debug=False)

    coords = nc.dram_tensor("coords", [S, D], fp32, kind="ExternalInput")
    out_dist = nc.dram_tensor("out_dist", [S, K1], fp32, kind="ExternalOutput")
    out_idx = nc.dram_tensor("out_idx", [S, K1], i32, kind="ExternalOutput")

    with TileContext(nc) as tc:
        with (
            tc.tile_pool(name="const", bufs=1) as cpool,
            tc.tile_pool(name="cand", bufs=4) as candpool,
            tc.tile_pool(name="small", bufs=8) as spool,
            tc.tile_pool(name="outs", bufs=3) as opool,
            tc.tile_pool(name="psum", bufs=4, space="PSUM") as ppool,
            tc.tile_pool(name="psumT", bufs=4, space="PSUM") as ptpool,
        ):
            # ---------------- persistent tensors ----------------
            rhs8 = cpool.tile([8, S], fp32)     # rows 0-3: c^T, rows 4-7: (c^T)^2
            lhsT8 = cpool.tile([8, S], fp32)    # rows 0-3: 2*c^T, rows 4-7: -1
            ident = cpool.tile([P, P], fp32)    # identity for PE transpose
            sqr_all = cpool.tile([P, NT], fp32) # -|c_r|^2 per row, per tile col
            nkA = cpool.tile([P, S], fp32)      # negkey/packed-key ping-pong
            nkB = cpool.tile([P, S], fp32)
            loc8 = cpool.tile([P, S], u8)       # j mod 128, as bytes
            slotb = cpool.tile([P, CW], u8)     # q (C-slot iota), as bytes
            goff = cpool.tile([P, CW], i16)     # 128 * (q >> 3)
            kio1 = cpool.tile([P, RW], i16)     # 1..72
            negone = cpool.tile([P, 1], fp32)   # -1.0 bias for rank decrement
            lscale = cpool.tile([8, 1], fp32)   # rows 0-3: 2.0, rows 4-7: 0.0
            lbias = cpool.tile([8, 1], fp32)    # rows 0-3: 0.0, rows 4-7: -1.0

            nc.vector.memset(negone, -1.0)
            # per-row scale/bias used to build lhsT8 in one activation
            nc.vector.memset(lscale, 2.0)
            nc.gpsimd.affine_select(lscale, lscale, [[0, 1]], Alu.is_ge, 0.0,
                                    base=3, channel_multiplier=-1)
            nc.vector.memset(lbias, -1.0)
            nc.gpsimd.affine_select(lbias, lbias, [[0, 1]], Alu.is_ge, 0.0,
                                    base=-4, channel_multiplier=1)

            # identity matrix: ones masked to the diagonal
            nc.vector.memset(ident, 1.0)
            nc.gpsimd.affine_select(
                ident, ident, [[1, P]], Alu.is_equal, 0.0,
                base=0, channel_multiplier=-1,
            )
            nc.gpsimd.iota(loc8, [[0, NG], [1, GW]], base=0,
                           channel_multiplier=0,
                           allow_small_or_imprecise_dtypes=True)
            nc.gpsimd.iota(slotb, [[1, CW]], base=0, channel_multiplier=0,
                           allow_small_or_imprecise_dtypes=True)
            nc.gpsimd.iota(goff, [[GW, NG], [0, 8]], base=0, channel_multiplier=0)
            nc.gpsimd.iota(kio1, [[1, RW]], base=1, channel_multiplier=0)

            # ---------------- prologue: build c^T layout ----------------
            # (engine APs must start at partition 0, so lhsT8 is written in
            # ONE activation whose per-row scale/bias produce 2*c^T on rows
            # 0-3 and -1 on rows 4-7.)
            for t4 in range(NT // 4):
                # one [8, 512] PSUM tile (a single bank) collects four
                # tiles' transposes so the rhs8/lhsT8 writes batch 4x.
                pT4 = ptpool.tile([2 * D, 4 * P], fp32, tag="pT4")
                ct32 = spool.tile([P, 8 * D], fp32, tag="ct32")
                cview = ct32.rearrange("p (k two d) -> p k two d", k=4, two=2)
                for k in range(4):
                    t = 4 * t4 + k
                    # cols 8k..8k+3 <- coords rows; 8k+4..8k+7 <- squares
                    nc.sync.dma_start(ct32[:, 8 * k:8 * k + D],
                                      coords[t * P:(t + 1) * P, :])
                nc.vector.tensor_tensor(out=cview[:, :, 1, :],
                                        in0=cview[:, :, 0, :],
                                        in1=cview[:, :, 0, :], op=Alu.mult)
                # -|c_r|^2 for these four tiles' 4x128 rows
                nc.vector.tensor_reduce(
                    sqr_all[:, 4 * t4:4 * t4 + 4], cview[:, :, 1, :],
                    axis=mybir.AxisListType.X, op=Alu.add, negate=True,
                )
                for k in range(4):
                    # transpose [128, 8] -> [8, 128]
                    nc.tensor.transpose(pT4[:, k * P:(k + 1) * P],
                                        ct32[:, 8 * k:8 * k + 2 * D], ident)
                cs4 = slice(4 * t4 * P, (4 * t4 + 4) * P)
                nc.vector.tensor_copy(rhs8[:, cs4], pT4)
                nc.scalar.activation(lhsT8[:, cs4], pT4, Act.Identity,
                                     bias=lbias, scale=lscale)

            # local_scatter lives in gpsimd ucode library 7; load it once.
            # (iota/affine_select above ran in the default library; the
            # tensor_scalar / scalar_tensor_tensor ops below are built-ins.)
            nc.gpsimd.load_library(library_config.local_scatter)

            # ---------------- main loop over row tiles ----------------
            # Software-pipelined emission: the DVE executes its queue in
            # order, and the C phase is a serial max8/match_replace chain.
            # Emitting tile t's (independent) group-phase ops interleaved
            # with tile t-1's C-phase ops lets the engine fill the chain's
            # dependency bubbles with useful work.

            def emit_cphase(st):
                """One DVE op of the pending tile's C phase; True if any."""
                r = st["round"]
                if r >= NR:
                    return False
                C2, V = st["C2"], st["V"]
                v8 = V[:, 8 * r:8 * r + 8]
                if not st["did_max"]:
                    nc.vector.max(v8, C2)
                    st["did_max"] = True
                    if r + 1 >= NR:
                        st["round"] = NR
                    return True
                nc.vector.match_replace(C2, v8, C2, NEG_BIG)
                st["did_max"] = False
                st["round"] = r + 1
                return True

            def emit_recovery(st):
                """Column unscramble + outputs for the pending tile."""
                V, colf, cs = st["V"], st["colf"], st["cs"]
                slot72 = spool.tile([P, RW], i16, tag="slot72")
                nc.scalar.activation(slot72, V.bitcast(u8)[:, 0:4 * RW:4],
                                     Act.Copy)
                W = spool.tile([P, CW], i16, tag="W")
                nc.gpsimd.local_scatter(W, kio1, slot72,
                                        channels=P, num_elems=CW, num_idxs=RW)
                Wm = spool.tile([P, CW], i16, tag="Wm")
                nc.vector.tensor_scalar_add(Wm, W, -1)
                Y = spool.tile([P, RW], i16, tag="Y")
                nc.gpsimd.local_scatter(Y, colf, Wm,
                                        channels=P, num_elems=RW, num_idxs=CW)
                dist65 = opool.tile([P, K1], fp32, tag="dist65")
                idx65 = opool.tile([P, K1], i32, tag="idx65")
                nc.scalar.activation(dist65, V[:, :K1], Act.Relu, scale=-1.0)
                nc.vector.tensor_copy(idx65, Y[:, :K1])
                nc.sync.dma_start(out_dist[cs, :], dist65)
                nc.sync.dma_start(out_idx[cs, :], idx65)

            pending = None
            carry = None
            for t in range(NT):
                cs = slice(t * P, (t + 1) * P)
                nk = nkA if t % 2 == 0 else nkB
                for h in range(S // HB):
                    ps = ppool.tile([P, HB], fp32, tag="ps")
                    for m in range(HB // 512):
                        col0 = h * HB + m * 512
                        nc.tensor.matmul(
                            ps[:, m * 512:(m + 1) * 512],
                            lhsT8[:, cs],
                            rhs8[:, col0:col0 + 512],
                            start=True, stop=True,
                        )
                    # negkey = psum - |c_r|^2
                    nc.scalar.activation(
                        nk[:, h * HB:(h + 1) * HB], ps,
                        Act.Identity, bias=sqr_all[:, t:t + 1],
                    )
                    # pack keys in place, chunk by chunk: the low byte of
                    # each fp32 negkey is overwritten with (j mod 128),
                    # leaving the value's top 24 bits as the sort key and
                    # the local column as tiebreaker.
                    for qq in range(2):
                        c0 = h * HB + qq * (HB // 2)
                        c1 = c0 + HB // 2
                        nc.gpsimd.tensor_copy(
                            nk.bitcast(u8)[:, 4 * c0:4 * c1:4],
                            loc8[:, c0:c1])

                # ---- group phase (tile t) + C phase (tile t-1) ----
                # After every 4 groups, ScalarE extracts those candidates'
                # local-column bytes into locb and then overwrites the same
                # bytes with the C-slot index -- so Cv is fully slot-coded
                # (and ready for its own C phase) moments after the last
                # Max8, with no bulk repack on the critical path.
                Cv = candpool.tile([P, CW], fp32, tag="Cv")
                locb = spool.tile([P, CW], i16, tag="locb")
                colf = spool.tile([P, CW], i16, tag="colf")
                Cvb = Cv.bitcast(u8)
                for g in range(NG):
                    pass
                    nc.vector.max(Cv[:, 8 * g:8 * g + 8],
                                  nk[:, g * GW:(g + 1) * GW])
                    if pending is not None and \
                            (g % 2 == 1 if t < NT - 1 else g % 4 == 1):
                        emit_cphase(pending)
                    if pending is not None and g == 1 and not pending["colf_done"]:
                        nc.vector.tensor_tensor(out=pending["colf"],
                                                in0=pending["locb"],
                                                in1=goff, op=Alu.add)
                        pending["colf_done"] = True
                    if g % 4 == 3:
                        ch = slice(8 * (g - 3), 8 * (g + 1))
                        bs = slice(32 * (g - 3), 4 * 8 * (g + 1), 4)
                        nc.scalar.activation(locb[:, ch], Cvb[:, bs],
                                             Act.Copy)
                        nc.scalar.activation(Cvb[:, bs], slotb[:, ch],
                                             Act.Copy)
                if pending is not None:
                    if t < NT - 1:
                        while emit_cphase(pending):
                            pass
                        emit_recovery(pending)
                    else:
                        carry = pending

                V = spool.tile([P, RW], fp32, tag="V")
                pending = {"C2": Cv, "V": V, "locb": locb, "colf": colf,
                           "cs": cs, "round": 0, "did_max": False,
                           "colf_done": False}

            # drain: the last two tiles' C phases are both still open
            # (the second-to-last was paced sparsely through the final group
            # phase) -- alternate their serial chains so each one's
            # dependency latency is hidden by the other's ops.
            nc.vector.tensor_tensor(out=pending["colf"], in0=pending["locb"],
                                    in1=goff, op=Alu.add)
            while True:
                a = emit_cphase(carry) if carry is not None else False
                b = emit_cphase(pending)
                if not (a or b):
                    break
            if carry is not None:
                emit_recovery(carry)
            emit_recovery(pending)

    nc.finalize()
    return nc


def _get_nc():
    if "nc" not in _NC_CACHE:
        _NC_CACHE["nc"] = _build_nc()
    return _NC_CACHE["nc"]


def _numpy_fallback(coordinates, row_splits):
    """Pure-numpy replica of the reference (used only on unexpected shapes)."""
    nB = int(row_splits.shape[0] - 1)
    N, nD = coordinates.shape
    nS = N // nB
    c = coordinates.reshape(nB, nS, nD).astype(np.float32)
    sq = np.sum(c * c, axis=-1)
    d2 = sq[:, :, None] + sq[:, None, :] - 2.0 * np.einsum(
        "bsd,btd->bst", c, c)
    d2 = np.maximum(d2, 0.0).astype(np.float32)
    k1 = min(K1, nS)
    idx = np.argsort(d2, axis=-1, kind="stable")[:, :, :k1]
    dist = np.take_along_axis(d2, idx, axis=-1)
    idx = idx + (np.arange(nB, dtype=np.int32) * nS)[:, None, None]
    return (idx.reshape(N, k1).astype(np.int32),
            dist.reshape(N, k1).astype(np.float32))


def kernel(coordinates, row_splits):
    coordinates = np.ascontiguousarray(coordinates, dtype=np.float32)
    rs = np.asarray(row_splits)
    expected_rs = np.arange(B + 1, dtype=np.int64) * S
    if coordinates.shape != (B * S, D) or rs.shape != (B + 1,) or \
            not np.array_equal(rs.astype(np.int64), expected_rs):
        return _numpy_fallback(coordinates, rs)

    from concourse import bass_utils

    nc = _get_nc()
    in_maps = [
        {"coords": coordinates[b * S:(b + 1) * S]} for b in range(B)
    ]
    res = bass_utils.run_bass_kernel_spmd(nc, in_maps, core_ids=list(range(B)))
    idx = np.concatenate(
        [res.results[b]["out_idx"] + np.int32(b * S) for b in range(B)], axis=0
    ).astype(np.int32)
    dist = np.concatenate(
        [res.results[b]["out_dist"] for b in range(B)], axis=0
    ).astype(np.float32)
    return idx, dist
